# revision 8
# baseline (speedup 1.0000x reference)
"""MoE location-expert router kernel for Trainium2 (8 NeuronCores).

Problem: out[i] = W[ptr[i] % 8] @ x[i] + b[ptr[i] % 8]
  x  [4096, 1024] f32, W [8, 32000, 1024] f32, b [8, 32000] f32 (zeros)
  out [4096, 32000] f32

Strategy (vocab / tensor-parallel sharding):
  - Host routes tokens: sort by expert, pad each expert group to a
    multiple of 128 (PSUM partition tile).
  - Each of the 8 cores owns a 4000-wide slice of the vocab dim of ALL
    8 experts -> identical SPMD program on every core, perfectly load
    balanced regardless of the routing distribution.
  - Per core: for each expert group of tokens, dense GEMM
    [n_e, 1024] @ [1024, 4000] accumulated over 8 K-chunks of 128 in
    PSUM, fp32r matmuls (1 cycle/row on PE vs 4 for plain fp32).
  - Host scatters the 8 x [4096, 4000] results back to original token
    order / full vocab.
"""

import math
import os

import numpy as np

import concourse.bacc as bacc
import concourse.bass as bass
import concourse.mybir as mybir
import concourse.tile as tile
from concourse.bass_utils import run_bass_kernel_spmd

E = 8          # experts
D = 1024       # d_model
V = 32000      # vocab
B = 4096       # tokens
NCORES = 8
VS = V // NCORES       # vocab slice per core (4000)
KT = 128               # contraction tile (partition dim)
KC = D // KT           # 8 K-chunks
MT = 128               # token tile (PSUM partition dim)
NT = 500               # vocab tile (moving free dim, <=512 for one PSUM bank)
NV = VS // NT          # 8 vocab tiles per core

# "f32r": fp32 storage, float32r matmul (fast PE path, ~fp32 accuracy)
# "bf16": bf16 storage for x/W (half the weight DMA), fp32 PSUM accum
MODE = os.environ.get("KERNEL_MODE", "f32r")

_program_cache = {}


def _build_program(pad_counts, counts, mode):
    """Trace the SPMD Tile program for the given per-expert padded counts."""
    m_total = int(sum(pad_counts))
    if mode == "bf16":
        io_dt = mybir.dt.bfloat16
    else:
        # float32r: same 4-byte layout as f32, PE runs 1 cycle/row
        # (vs 4 for plain f32). Verifier requires producer+consumer
        # dtype to be f32r end-to-end, so declare DRAM+SBUF as f32r.
        io_dt = mybir.dt.float32r

    nc = bacc.Bacc("TRN2", target_bir_lowering=False, debug=False,
                   enable_asserts=False, num_devices=NCORES)

    xT = nc.dram_tensor("xT", [D, m_total], io_dt, kind="ExternalInput").ap()
    wT = nc.dram_tensor("wT", [E, D, VS], io_dt, kind="ExternalInput").ap()
    out = nc.dram_tensor("out", [B, VS], mybir.dt.float32,
                         kind="ExternalOutput").ap()

    # [ (kc p) m -> p kc m ] views for K-chunked loads
    xT_r = xT.rearrange("(kc p) m -> p kc m", p=KT)

    with tile.TileContext(nc) as tc:
        with (
            tc.tile_pool(name="xp", bufs=2) as xpool,
            tc.tile_pool(name="wp", bufs=3) as wpool,
            tc.tile_pool(name="op", bufs=4) as opool,
            tc.tile_pool(name="ps", bufs=8, space="PSUM") as pspool,
        ):
            pad_off = 0   # column offset in padded xT
            val_off = 0   # row offset in valid-packed out
            for e in range(E):
                pe = int(pad_counts[e])
                if pe == 0:
                    continue
                # number of valid (un-padded) rows for this expert is
                # reconstructed by the caller; here we only need tile count
                xe = xpool.tile([KT, KC, pe], io_dt, tag="x")
                nc.sync.dma_start(
                    out=xe[:, :, :],
                    in_=xT_r[:, :, pad_off:pad_off + pe],
                )
                wT_e = wT[e].rearrange("(kc p) v -> p kc v", p=KT)
                for v in range(NV):
                    wt = wpool.tile([KT, KC, NT], io_dt, tag="w")
                    nc.sync.dma_start(
                        out=wt[:, :, :],
                        in_=wT_e[:, :, v * NT:(v + 1) * NT],
                    )
                    for t in range(pe // MT):
                        pt = pspool.tile([MT, NT], mybir.dt.float32, tag="ps")
                        for kc in range(KC):
                            lhsT = xe[:, kc, t * MT:(t + 1) * MT]
                            rhs = wt[:, kc, :]
                            nc.tensor.matmul(
                                pt[:, :], lhsT, rhs,
                                start=(kc == 0), stop=(kc == KC - 1),
                            )
                        ot = opool.tile([MT, NT], mybir.dt.float32, tag="o")
                        nc.vector.tensor_copy(ot[:, :], pt[:, :])
                        # only write valid rows (counts may not fill last tile)
                        valid = min(MT, int(counts[e]) - t * MT)
                        nc.sync.dma_start(
                            out=out[val_off + t * MT:val_off + t * MT + valid,
                                    v * NT:(v + 1) * NT],
                            in_=ot[:valid, :],
                        )
                pad_off += pe
                val_off += int(counts[e])
    nc.compile()
    return nc, m_total


def _get_program(counts, mode):
    pad_counts = tuple(int(-(-c // MT) * MT) for c in counts)
    key = (pad_counts, tuple(int(c) for c in counts), mode)
    if key not in _program_cache:
        _program_cache[key] = _build_program(pad_counts, counts, mode)
    return pad_counts, _program_cache[key]


def _prepare(x, pointer_addresses, W, mode):
    idx = (np.asarray(pointer_addresses).astype(np.int64) % E).astype(np.int32)
    counts = np.bincount(idx, minlength=E)
    order = np.argsort(idx, kind="stable")
    pad_counts, (nc, m_total) = _get_program(tuple(counts), mode)

    np_dt = np.dtype("float32")
    if mode == "bf16":
        import ml_dtypes
        np_dt = np.dtype(ml_dtypes.bfloat16)

    x = np.asarray(x, dtype=np.float32)
    xs = x[order]                      # [B, D] sorted by expert
    x_pad = np.zeros((m_total, D), dtype=np_dt)
    row = 0
    srow = 0
    for e in range(E):
        c = int(counts[e])
        x_pad[row:row + c] = xs[srow:srow + c]
        row += int(pad_counts[e])
        srow += c
    xT = np.ascontiguousarray(x_pad.T)  # [D, m_total]

    W = np.asarray(W)
    wts = []
    for c in range(NCORES):
        Wc = W[:, c * VS:(c + 1) * VS, :]                 # [E, VS, D] view
        WTc = np.ascontiguousarray(Wc.transpose(0, 2, 1))  # [E, D, VS]
        if mode == "bf16":
            WTc = WTc.astype(np_dt)
        wts.append(WTc)
    return idx, order, nc, xT, wts


def _run(x, pointer_addresses, W, b, trace=False, mode=None):
    mode = mode or MODE
    idx, order, nc, xT, wts = _prepare(x, pointer_addresses, W, mode)
    in_maps = [{"xT": xT, "wT": wts[c]} for c in range(NCORES)]
    kw = {}
    if trace:
        kw = dict(trace=True, trace_cores=[0])
    res = run_bass_kernel_spmd(nc, in_maps, list(range(NCORES)), **kw)

    out = np.empty((B, V), dtype=np.float32)
    for c in range(NCORES):
        out[order, c * VS:(c + 1) * VS] = res.results[c]["out"]

    b = np.asarray(b)
    if b.any():
        for e in range(E):
            out[idx == e] += b[e].astype(np.float32)
    return out, res


def kernel(x, pointer_addresses, W, b):
    out, _ = _run(x, pointer_addresses, W, b, trace=False)
    return out


# revision 12
# speedup vs baseline: 1.1342x; 1.1342x over previous
"""MoE location-expert router kernel for Trainium2 (8 NeuronCores).

Problem: out[i] = W[ptr[i] % 8] @ x[i] + b[ptr[i] % 8]
  x  [4096, 1024] f32, W [8, 32000, 1024] f32, b [8, 32000] f32 (zeros)
  out [4096, 32000] f32

Strategy (vocab / tensor-parallel sharding):
  - Host routes tokens: sort by expert, pad each expert group to a
    multiple of 128 (PSUM partition tile).
  - Each of the 8 cores owns a 4000-wide slice of the vocab dim of ALL
    8 experts -> identical SPMD program on every core, perfectly load
    balanced regardless of the routing distribution.
  - Per core: for each expert group of tokens, dense GEMM
    [n_e, 1024] @ [1024, 4000] accumulated over 8 K-chunks of 128 in
    PSUM, fp32r matmuls (1 cycle/row on PE vs 4 for plain fp32).
  - Host scatters the 8 x [4096, 4000] results back to original token
    order / full vocab.
"""

import math
import os

import numpy as np

import concourse.bacc as bacc
import concourse.bass as bass
import concourse.mybir as mybir
import concourse.tile as tile
from concourse.bass_utils import run_bass_kernel_spmd

E = 8          # experts
D = 1024       # d_model
V = 32000      # vocab
B = 4096       # tokens
NCORES = 8
VS = V // NCORES       # vocab slice per core (4000)
KT = 128               # contraction tile (partition dim)
KC = D // KT           # 8 K-chunks
MT = 128               # token tile (PSUM partition dim)
NT = 500               # vocab tile (moving free dim, <=512 for one PSUM bank)
NV = VS // NT          # 8 vocab tiles per core

# "f32r": fp32 storage, float32r matmul (fast PE path, ~fp32 accuracy)
# "fp16": fp16 storage for x/W (half the weight DMA, fast FWL weight
#         loads that overlap with matmuls), fp32 PSUM accum
# "bf16": like fp16 but bfloat16 (worse mantissa, kept for comparison)
MODE = os.environ.get("KERNEL_MODE", "fp16")

_program_cache = {}


def _build_program(pad_counts, counts, mode):
    """Trace the SPMD Tile program for the given per-expert padded counts."""
    m_total = int(sum(pad_counts))
    if mode == "fp16":
        io_dt = mybir.dt.float16
    elif mode == "bf16":
        io_dt = mybir.dt.bfloat16
    else:
        # float32r: same 4-byte layout as f32, PE runs 1 cycle/row
        # (vs 4 for plain f32). Verifier requires producer+consumer
        # dtype to be f32r end-to-end, so declare DRAM+SBUF as f32r.
        io_dt = mybir.dt.float32r

    nc = bacc.Bacc("TRN2", target_bir_lowering=False, debug=False,
                   enable_asserts=False, num_devices=NCORES)

    xT = nc.dram_tensor("xT", [D, m_total], io_dt, kind="ExternalInput").ap()
    wT = nc.dram_tensor("wT", [E, D, VS], io_dt, kind="ExternalInput").ap()
    out = nc.dram_tensor("out", [B, VS], mybir.dt.float32,
                         kind="ExternalOutput").ap()

    # [ (kc p) m -> p kc m ] views for K-chunked loads
    xT_r = xT.rearrange("(kc p) m -> p kc m", p=KT)

    with tile.TileContext(nc) as tc:
        with (
            tc.tile_pool(name="xp", bufs=2) as xpool,
            tc.tile_pool(name="wp", bufs=3) as wpool,
            tc.tile_pool(name="op", bufs=4) as opool,
            tc.tile_pool(name="ps", bufs=8, space="PSUM") as pspool,
        ):
            pad_off = 0   # column offset in padded xT
            val_off = 0   # row offset in valid-packed out
            for e in range(E):
                pe = int(pad_counts[e])
                if pe == 0:
                    continue
                # number of valid (un-padded) rows for this expert is
                # reconstructed by the caller; here we only need tile count
                xe = xpool.tile([KT, KC, pe], io_dt, tag="x")
                nc.sync.dma_start(
                    out=xe[:, :, :],
                    in_=xT_r[:, :, pad_off:pad_off + pe],
                )
                wT_e = wT[e].rearrange("(kc p) v -> p kc v", p=KT)
                for v in range(NV):
                    wt = wpool.tile([KT, KC, NT], io_dt, tag="w")
                    nc.sync.dma_start(
                        out=wt[:, :, :],
                        in_=wT_e[:, :, v * NT:(v + 1) * NT],
                    )
                    for t in range(pe // MT):
                        pt = pspool.tile([MT, NT], mybir.dt.float32, tag="ps")
                        for kc in range(KC):
                            lhsT = xe[:, kc, t * MT:(t + 1) * MT]
                            rhs = wt[:, kc, :]
                            nc.tensor.matmul(
                                pt[:, :], lhsT, rhs,
                                start=(kc == 0), stop=(kc == KC - 1),
                            )
                        ot = opool.tile([MT, NT], mybir.dt.float32, tag="o")
                        nc.vector.tensor_copy(ot[:, :], pt[:, :])
                        # only write valid rows (counts may not fill last tile)
                        valid = min(MT, int(counts[e]) - t * MT)
                        nc.sync.dma_start(
                            out=out[val_off + t * MT:val_off + t * MT + valid,
                                    v * NT:(v + 1) * NT],
                            in_=ot[:valid, :],
                        )
                pad_off += pe
                val_off += int(counts[e])
    nc.compile()
    return nc, m_total


def _get_program(counts, mode):
    pad_counts = tuple(int(-(-c // MT) * MT) for c in counts)
    key = (pad_counts, tuple(int(c) for c in counts), mode)
    if key not in _program_cache:
        _program_cache[key] = _build_program(pad_counts, counts, mode)
    return pad_counts, _program_cache[key]


def _prepare(x, pointer_addresses, W, mode):
    idx = (np.asarray(pointer_addresses).astype(np.int64) % E).astype(np.int32)
    counts = np.bincount(idx, minlength=E)
    order = np.argsort(idx, kind="stable")
    pad_counts, (nc, m_total) = _get_program(tuple(counts), mode)

    np_dt = np.dtype("float32")
    if mode == "fp16":
        np_dt = np.dtype(np.float16)
    elif mode == "bf16":
        import ml_dtypes
        np_dt = np.dtype(ml_dtypes.bfloat16)

    x = np.asarray(x, dtype=np.float32)
    xs = x[order]                      # [B, D] sorted by expert
    x_pad = np.zeros((m_total, D), dtype=np_dt)
    row = 0
    srow = 0
    for e in range(E):
        c = int(counts[e])
        x_pad[row:row + c] = xs[srow:srow + c]
        row += int(pad_counts[e])
        srow += c
    xT = np.ascontiguousarray(x_pad.T)  # [D, m_total]

    W = np.asarray(W)
    wts = []
    for c in range(NCORES):
        Wc = W[:, c * VS:(c + 1) * VS, :]                 # [E, VS, D] view
        WTc = np.ascontiguousarray(Wc.transpose(0, 2, 1))  # [E, D, VS]
        if mode in ("fp16", "bf16"):
            WTc = WTc.astype(np_dt)
        wts.append(WTc)
    return idx, order, nc, xT, wts


def _run(x, pointer_addresses, W, b, trace=False, mode=None):
    mode = mode or MODE
    idx, order, nc, xT, wts = _prepare(x, pointer_addresses, W, mode)
    in_maps = [{"xT": xT, "wT": wts[c]} for c in range(NCORES)]
    kw = {}
    if trace:
        kw = dict(trace=True, trace_cores=[0])
    res = run_bass_kernel_spmd(nc, in_maps, list(range(NCORES)), **kw)

    out = np.empty((B, V), dtype=np.float32)
    for c in range(NCORES):
        out[order, c * VS:(c + 1) * VS] = res.results[c]["out"]

    b = np.asarray(b)
    if b.any():
        for e in range(E):
            out[idx == e] += b[e].astype(np.float32)
    return out, res


def kernel(x, pointer_addresses, W, b):
    out, _ = _run(x, pointer_addresses, W, b, trace=False)
    return out


# revision 14
# speedup vs baseline: 1.3118x; 1.1566x over previous
"""MoE location-expert router kernel for Trainium2 (8 NeuronCores).

Problem: out[i] = W[ptr[i] % 8] @ x[i] + b[ptr[i] % 8]
  x  [4096, 1024] f32, W [8, 32000, 1024] f32, b [8, 32000] f32 (zeros)
  out [4096, 32000] f32

Strategy (vocab / tensor-parallel sharding):
  - Host routes tokens: sort by expert, pad each expert group to a
    multiple of 128 (PSUM partition tile).
  - Each of the 8 cores owns a 4000-wide slice of the vocab dim of ALL
    8 experts -> identical SPMD program on every core, perfectly load
    balanced regardless of the routing distribution.
  - Per core: for each expert group of tokens, dense GEMM
    [n_e, 1024] @ [1024, 4000] accumulated over 8 K-chunks of 128 in
    PSUM, fp32r matmuls (1 cycle/row on PE vs 4 for plain fp32).
  - Host scatters the 8 x [4096, 4000] results back to original token
    order / full vocab.
"""

import os

import numpy as np

import concourse.bacc as bacc
import concourse.bass as bass
import concourse.mybir as mybir
import concourse.tile as tile
from concourse.bass_utils import run_bass_kernel_spmd

E = 8          # experts
D = 1024       # d_model
V = 32000      # vocab
B = 4096       # tokens
NCORES = 8
VS = V // NCORES       # vocab slice per core (4000)
KT = 128               # contraction tile (partition dim)
KC = D // KT           # 8 K-chunks
MT = 128               # token tile (PSUM partition dim)
NT = 500               # vocab tile (moving free dim, <=512 for one PSUM bank)
NV = VS // NT          # 8 vocab tiles per core

# "f32r": fp32 storage, float32r matmul (fast PE path, ~fp32 accuracy)
# "fp16": fp16 storage for x/W (half the weight DMA, fast FWL weight
#         loads that overlap with matmuls), fp32 PSUM accum
# "bf16": like fp16 but bfloat16 (worse mantissa, kept for comparison)
MODE = os.environ.get("KERNEL_MODE", "fp16")

_program_cache = {}


def _build_program(pad_counts, counts, mode):
    """Trace the SPMD Tile program for the given per-expert padded counts."""
    m_total = int(sum(pad_counts))
    if mode == "fp16":
        io_dt = mybir.dt.float16
    elif mode == "bf16":
        io_dt = mybir.dt.bfloat16
    else:
        # float32r: same 4-byte layout as f32, PE runs 1 cycle/row
        # (vs 4 for plain f32). Verifier requires producer+consumer
        # dtype to be f32r end-to-end, so declare DRAM+SBUF as f32r.
        io_dt = mybir.dt.float32r

    nc = bacc.Bacc("TRN2", target_bir_lowering=False, debug=False,
                   enable_asserts=False, num_devices=NCORES)

    xT = nc.dram_tensor("xT", [D, m_total], io_dt, kind="ExternalInput").ap()
    wT = nc.dram_tensor("wT", [E, D, VS], io_dt, kind="ExternalInput").ap()
    out = nc.dram_tensor("out", [B, VS], mybir.dt.float32,
                         kind="ExternalOutput").ap()

    # [ (kc p) m -> p kc m ] views for K-chunked loads
    xT_r = xT.rearrange("(kc p) m -> p kc m", p=KT)

    with tile.TileContext(nc) as tc:
        with (
            tc.tile_pool(name="xp", bufs=2) as xpool,
            tc.tile_pool(name="wp", bufs=6) as wpool,
            tc.tile_pool(name="op", bufs=6) as opool,
            tc.tile_pool(name="ps", bufs=8, space="PSUM") as pspool,
        ):
            pad_off = 0   # column offset in padded xT
            val_off = 0   # row offset in valid-packed out
            for e in range(E):
                pe = int(pad_counts[e])
                if pe == 0:
                    continue
                # number of valid (un-padded) rows for this expert is
                # reconstructed by the caller; here we only need tile count
                xe = xpool.tile([KT, KC, pe], io_dt, tag="x")
                nc.sync.dma_start(
                    out=xe[:, :, :],
                    in_=xT_r[:, :, pad_off:pad_off + pe],
                )
                wT_e = wT[e].rearrange("(kc p) v -> p kc v", p=KT)
                for v in range(NV):
                    wt = wpool.tile([KT, KC, NT], io_dt, tag="w")
                    nc.sync.dma_start(
                        out=wt[:, :, :],
                        in_=wT_e[:, :, v * NT:(v + 1) * NT],
                    )
                    for t in range(pe // MT):
                        pt = pspool.tile([MT, NT], mybir.dt.float32, tag="ps")
                        for kc in range(KC):
                            lhsT = xe[:, kc, t * MT:(t + 1) * MT]
                            rhs = wt[:, kc, :]
                            nc.tensor.matmul(
                                pt[:, :], lhsT, rhs,
                                start=(kc == 0), stop=(kc == KC - 1),
                            )
                        ot = opool.tile([MT, NT], mybir.dt.float32, tag="o")
                        nc.vector.tensor_copy(ot[:, :], pt[:, :])
                        # only write valid rows (counts may not fill last tile)
                        valid = min(MT, int(counts[e]) - t * MT)
                        nc.sync.dma_start(
                            out=out[val_off + t * MT:val_off + t * MT + valid,
                                    v * NT:(v + 1) * NT],
                            in_=ot[:valid, :],
                        )
                pad_off += pe
                val_off += int(counts[e])
    nc.compile()
    return nc, m_total


def _get_program(counts, mode):
    pad_counts = tuple(int(-(-c // MT) * MT) for c in counts)
    key = (pad_counts, tuple(int(c) for c in counts), mode)
    if key not in _program_cache:
        _program_cache[key] = _build_program(pad_counts, counts, mode)
    return pad_counts, _program_cache[key]


def _prepare(x, pointer_addresses, W, mode):
    idx = (np.asarray(pointer_addresses).astype(np.int64) % E).astype(np.int32)
    counts = np.bincount(idx, minlength=E)
    order = np.argsort(idx, kind="stable")
    pad_counts, (nc, m_total) = _get_program(tuple(counts), mode)

    np_dt = np.dtype("float32")
    if mode == "fp16":
        np_dt = np.dtype(np.float16)
    elif mode == "bf16":
        import ml_dtypes
        np_dt = np.dtype(ml_dtypes.bfloat16)

    x = np.asarray(x, dtype=np.float32)
    xs = x[order]                      # [B, D] sorted by expert
    x_pad = np.zeros((m_total, D), dtype=np_dt)
    row = 0
    srow = 0
    for e in range(E):
        c = int(counts[e])
        x_pad[row:row + c] = xs[srow:srow + c]
        row += int(pad_counts[e])
        srow += c
    xT = np.ascontiguousarray(x_pad.T)  # [D, m_total]

    W = np.asarray(W)
    wts = []
    for c in range(NCORES):
        Wc = W[:, c * VS:(c + 1) * VS, :]                 # [E, VS, D] view
        WTc = np.ascontiguousarray(Wc.transpose(0, 2, 1))  # [E, D, VS]
        if mode in ("fp16", "bf16"):
            WTc = WTc.astype(np_dt)
        wts.append(WTc)
    return idx, order, nc, xT, wts


def _run(x, pointer_addresses, W, b, trace=False, mode=None):
    mode = mode or MODE
    idx, order, nc, xT, wts = _prepare(x, pointer_addresses, W, mode)
    in_maps = [{"xT": xT, "wT": wts[c]} for c in range(NCORES)]
    kw = {}
    if trace:
        kw = dict(trace=True, trace_cores=[0])
    res = run_bass_kernel_spmd(nc, in_maps, list(range(NCORES)), **kw)

    out = np.empty((B, V), dtype=np.float32)
    for c in range(NCORES):
        out[order, c * VS:(c + 1) * VS] = res.results[c]["out"]

    b = np.asarray(b)
    if b.any():
        for e in range(E):
            out[idx == e] += b[e].astype(np.float32)
    return out, res


def kernel(x, pointer_addresses, W, b):
    out, _ = _run(x, pointer_addresses, W, b, trace=False)
    return out
